# revision 13
# baseline (speedup 1.0000x reference)
"""GQA attention kernel for Trainium2, 8-core tensor-parallel over kv heads.

Reference computation (fp32):
  q  = query @ q_proj.T + q_bias      -> heads (g-major): dq = gi*H*D + hi*D + d
  kv = query @ kv_proj.T + kv_bias    -> per kv head hi: k = cols [hi*2D, hi*2D+D), v = next D
  attn = softmax(q k^T / sqrt(D));  out = (attn v) @ out_proj.T + out_bias

Sharding: 8 cores; core c handles kv head h0 = c//2 and 4 query-head groups
gis = [0..3] (c even) or [4..7] (c odd). Each core computes a full-shape
partial of the output (rank-256 contribution); host sums the 8 partials.

Schedule (v5): evolution of the v4 aux-queue design guided by the ntff trace:
 - startup: qt loads lead the gpsimd queue; weights stream per-ec on the
   sync queue so the PE's first matmul issues ~9us in and the qt feed rate
   (1 DMA/650ns) always beats PE consumption (6 matmuls/ec).
 - P1 psum: pq0/pq1 evict on ACT+DVE in parallel; pkv double-buffered so
   tchunk boundaries don't WAR-stall the PE (HAM ramp resets cost 2x).
 - V' transposes moved off the PE onto the sync queue's DMA-transpose XBAR
   (frees the psT psum banks and ~12us of PE/DVE time).
 - tail: after the last attention unit the score/avA psum pools close and
   the remaining P3 tiles rotate over 6 fresh psum banks with evictions
   alternated between DVE and ACT, so the drain runs near PE speed instead
   of serializing on a 2-bank WAR round trip (the v4 tail was 94us at 67%
   PE busy with HAM k=4 half-clock windows).
 - n1 unit order (p0,h0),(p1,h0),(p0,h1),(p1,h1) so half0's normalization
   completes two units early and P3(n1,half0) rides the aux queue.
"""
import sys

sys.path.insert(0, "/opt/trn_rl_repo")

from collections import deque

import ml_dtypes
import numpy as np

import concourse.bass as bass
import concourse.mybir as mybir
import concourse.tile as tile
from concourse import bacc

H, G, D = 4, 8, 64
L, N, E = 2048, 2, 2048
T = N * L
P = 128
DQ = 256  # per-core q dim: 4 groups x 64
SCALE = float(D) ** -0.5
F32 = mybir.dt.float32
BF16 = mybir.dt.bfloat16


def pbcast(ap2d, p):
    """[1, F] AP -> [p, F] AP broadcast across partitions (stride 0)."""
    return bass.AP(tensor=ap2d.tensor, offset=ap2d.offset, ap=[[0, p]] + list(ap2d.ap[1:]))


class AuxQueue:
    """FIFO of ('mm'|'free', closure) events.  'mm' events are budgeted
    (one PE matmul each); 'free' events (DMAs, DVE evicts, allocs) are
    emitted alongside at no budget cost."""

    def __init__(self):
        self.q = deque()

    def push(self, events):
        self.q.extend(events)

    def push_front(self, events):
        self.q.extendleft(reversed(events))

    def pop(self, budget):
        n = 0
        while self.q and n < budget:
            kind, fn = self.q.popleft()
            fn()
            if kind == 'mm':
                n += 1

    def drain(self):
        while self.q:
            kind, fn = self.q.popleft()
            fn()


class AuxPool:
    """Round-robin over aux psum tags (1 bank each).  Items resolve the
    pool at event-execution time via this holder, so deferred events run
    against whichever block's pool is current."""

    def __init__(self, pool, tags):
        self.pool = pool
        self.tags = tags
        self.i = 0

    def tile(self):
        t = self.pool.tile([P, 512], F32, name="aux", tag=self.tags[self.i])
        self.i = (self.i + 1) % len(self.tags)
        return t


CUR = {}  # CUR['apool'] = the active AuxPool


def build_nc():
    nc = bacc.Bacc("TRN2", target_bir_lowering=False, debug=False)
    add = mybir.AluOpType.add
    ident_f = mybir.ActivationFunctionType.Identity

    qT = nc.dram_tensor("qT", [E, T], BF16, kind="ExternalInput").ap()
    qpT = nc.dram_tensor("qpT", [P, 16 * DQ], BF16, kind="ExternalInput").ap()
    kvpT = nc.dram_tensor("kvpT", [P, 16 * P], BF16, kind="ExternalInput").ap()
    opT = nc.dram_tensor("opT", [DQ, E], BF16, kind="ExternalInput").ap()
    qb = nc.dram_tensor("qb", [P, 2], F32, kind="ExternalInput").ap()
    kvb = nc.dram_tensor("kvb", [P, 1], F32, kind="ExternalInput").ap()
    ones16 = nc.dram_tensor("ones16", [P, 16], BF16, kind="ExternalInput").ap()
    out = nc.dram_tensor("out", [T, E], BF16, kind="ExternalOutput").ap()
    denombuf = nc.dram_tensor("denombuf", [1, 8 * 2048], F32, kind="Internal").ap()
    recipbuf = nc.dram_tensor("recipbuf", [1, 8 * 2048], BF16, kind="Internal").ap()

    with tile.TileContext(nc) as tc, tc.tile_pool(name="data", bufs=1) as data, \
            tc.tile_pool(name="consts", bufs=1) as consts, \
            tc.tile_pool(name="qload", bufs=10) as qload, \
            tc.tile_pool(name="qload2", bufs=18) as qload2, \
            tc.tile_pool(name="expA", bufs=4) as expA, \
            tc.tile_pool(name="expB", bufs=26) as expB, \
            tc.tile_pool(name="scratch", bufs=3) as scratch, \
            tc.tile_pool(name="ostage", bufs=7) as ostage:
        qpT_all = consts.tile([P, 16 * DQ], BF16)
        kvpT_all = consts.tile([P, 16 * P], BF16)
        qb_sb = consts.tile([P, 2], F32)
        kvb_sb = consts.tile([P, 1], F32)

        QT0 = data.tile([P, T], BF16)  # dq 0:128   (gi_loc 0, 1)
        QT1 = data.tile([P, T], BF16)  # dq 128:256 (gi_loc 2, 3)
        KVT = data.tile([P, T], BF16)  # k rows 0:64, v rows 64:128
        KTdup = data.tile([P, T], BF16)  # k rows duplicated at partitions 64:128
        attn0 = data.tile([P, T], BF16)  # attnoutT c-chunk 0 (gi_loc 0, 1)
        attn1 = data.tile([P, T], BF16)  # c-chunk 1 (gi_loc 2, 3)
        Vp = [data.tile([P, 16 * 80], BF16, name=f"vp{n}", tag=f"vp{n}") for n in range(N)]
        opT_sb = [data.tile([P, E], BF16, name=f"opt{cc}", tag=f"opt{cc}") for cc in range(2)]

        def wslice(target, ec):
            return (qpT_all[:, ec * DQ:ec * DQ + P],
                    qpT_all[:, ec * DQ + P:(ec + 1) * DQ],
                    kvpT_all[:, ec * P:(ec + 1) * P])[target]

        def load_weights(ec):
            nc.sync.dma_start(out=qpT_all[:, ec * DQ:(ec + 1) * DQ],
                              in_=qpT[:, ec * DQ:(ec + 1) * DQ])
            nc.sync.dma_start(out=kvpT_all[:, ec * P:(ec + 1) * P],
                              in_=kvpT[:, ec * P:(ec + 1) * P])

        def late_consts():
            for n in range(N):
                vcol = Vp[n].rearrange("p (m c) -> p m c", c=80)[:, :, 64:65]
                nc.sync.dma_start(out=vcol, in_=ones16)

        def load_opT():
            for cc in range(2):
                nc.sync.dma_start(out=opT_sb[cc][:], in_=opT[cc * P:(cc + 1) * P, :])

        def vp_transposes(lo, width):
            """DMA-transpose XBAR: V rows of KVT[64:128, lo:lo+width] into
            Vp chunks (keys-major [128,64] each)."""
            n = lo // L
            for k in range(width // P):
                mc = (lo - n * L) // P + k
                nc.sync.dma_start(
                    out=Vp[n][:, mc * 80:mc * 80 + 64],
                    in_=KVT[64:128, lo + k * P:lo + (k + 1) * P],
                    transpose=True)

        # ---------------- Phase 1 bulk ----------------
        def p1_tchunk(tchunk, ps1a, ps1kv, hook=None):
            tcols = slice(tchunk * 1024, (tchunk + 1) * 1024)
            pq0 = ps1a.tile([P, 1024], F32, tag="pq0")
            pq1 = ps1a.tile([P, 1024], F32, tag="pq1")
            pkv = ps1kv.tile([P, 1024], F32, tag="pkv")
            for ec in range(16):
                qt = qload.tile([P, 1024], BF16, tag="qt")
                nc.gpsimd.dma_start(out=qt[:], in_=qT[ec * P:(ec + 1) * P, tcols])
                if hook is not None:
                    hook(ec)
                first, last = ec == 0, ec == 15
                for ps_t, target in ((pkv, 2), (pq0, 0), (pq1, 1)):
                    w = wslice(target, ec)
                    for lq in range(2):
                        nc.tensor.matmul(ps_t[:, lq * 512:(lq + 1) * 512], lhsT=w,
                                         rhs=qt[:, lq * 512:(lq + 1) * 512],
                                         start=first, stop=last)
            # parallel evictions: ACT for pq0, DVE for pq1, ACT for pkv
            nc.scalar.activation(QT0[:, tcols], pq0[:], ident_f, bias=qb_sb[:, 0:1])
            nc.vector.tensor_scalar(QT1[:, tcols], pq1[:], qb_sb[:, 1:2], None, op0=add)
            nc.vector.tensor_scalar(KVT[:, tcols], pkv[:], kvb_sb[:, 0:1], None, op0=add)
            nc.sync.dma_start(out=KTdup[64:128, tcols], in_=KVT[0:64, tcols])
            vp_transposes(tchunk * 1024, 1024)

        # ---------------- aux items ----------------
        def item_p1_super(tchunk, colhalf):
            """All three projection targets over one 512-col span, sharing a
            single set of 16 qt loads (issued on the idle gpsimd queue).
            Order pkv, pq0, pq1 so KTdup/Vp are produced earliest."""
            lo = tchunk * 1024 + colhalf * 512
            qts = {}

            def ldq(ec):
                qts[ec] = qload2.tile([P, 512], BF16, name="qt2", tag="qt2")
                nc.gpsimd.dma_start(out=qts[ec][:],
                                    in_=qT[ec * P:(ec + 1) * P, lo:lo + 512])
            ev = []
            for ec in range(6):
                ev.append(('free', lambda ec=ec: ldq(ec)))
            for ti, target in enumerate((2, 0, 1)):
                cell = {}

                def alloc(cell=cell):
                    cell['t'] = CUR['apool'].tile()
                ev.append(('free', alloc))
                for ec in range(16):
                    if ti == 0 and ec + 6 < 16:
                        ev.append(('free', lambda ec=ec: ldq(ec + 6)))

                    def mm(ec=ec, target=target, cell=cell, last=ti == 2):
                        q = qts.pop(ec) if last else qts[ec]
                        nc.tensor.matmul(cell['t'][:], lhsT=wslice(target, ec),
                                         rhs=q[:], start=ec == 0, stop=ec == 15)
                    ev.append(('mm', mm))

                def evict(cell=cell, target=target):
                    dst = (QT0, QT1, KVT)[target]
                    bias = (qb_sb[:, 0:1], qb_sb[:, 1:2], kvb_sb[:, 0:1])[target]
                    nc.vector.tensor_scalar(dst[:, lo:lo + 512], cell['t'][:],
                                            bias, None, op0=add)
                    if target == 2:
                        nc.sync.dma_start(out=KTdup[64:128, lo:lo + 512],
                                          in_=KVT[0:64, lo:lo + 512])
                        vp_transposes(lo, 512)
                ev.append(('free', evict))
            return ev

        def item_avB(n, pair, half, lq, ebs):
            """Deferred head-B AV over staged exp tiles, one 512-col pass."""
            attnp = attn0 if pair == 0 else attn1
            lo = n * L + half * 1024 + lq * 512
            seg = ((n * 2 + pair) * 2 + half) * 2048
            cell = {}

            def alloc():
                cell['t'] = CUR['apool'].tile()
            ev = [('free', alloc)]
            for mc in range(16):
                def mm(mc=mc):
                    vw = Vp[n][:, mc * 80:mc * 80 + 65]
                    nc.tensor.matmul(cell['t'][0:65, :], lhsT=vw,
                                     rhs=ebs[mc][:, lq * 512:(lq + 1) * 512],
                                     start=mc == 0, stop=mc == 15)
                ev.append(('mm', mm))

            def evict():
                sc = scratch.tile([64, 512], BF16, name="sc", tag="sc")
                nc.vector.tensor_copy(sc[:], cell['t'][0:64, :])
                nc.sync.dma_start(out=attnp[64:128, lo:lo + 512], in_=sc[:])
                dnB = scratch.tile([1, 512], F32, name="dnB", tag="dnB")
                nc.vector.tensor_copy(dnB[:], cell['t'][64:65, :])
                dB = seg + 1024 + lq * 512
                nc.sync.dma_start(out=denombuf[0:1, dB:dB + 512], in_=dnB[:])
            ev.append(('free', evict))
            return ev

        def item_p3(tt, eo, lq, evict_eng='dve'):
            """Output-projection 512-col tile: rows tt*128, e cols eo*1024+lq*512."""
            trows = slice(tt * P, (tt + 1) * P)
            ecol = eo * 1024 + lq * 512
            cell = {}

            def alloc():
                cell['t'] = CUR['apool'].tile()
            ev = [('free', alloc)]
            for cc in range(2):
                def mm(cc=cc):
                    src = attn0 if cc == 0 else attn1
                    nc.tensor.matmul(cell['t'][:], lhsT=src[:, trows],
                                     rhs=opT_sb[cc][:, ecol:ecol + 512],
                                     start=cc == 0, stop=cc == 1)
                ev.append(('mm', mm))

            def evict():
                ost = ostage.tile([P, 512], BF16, name="ost", tag="ost")
                if evict_eng == 'dve':
                    nc.vector.tensor_copy(ost[:], cell['t'][:])
                else:
                    nc.scalar.copy(ost[:], cell['t'][:])
                nc.sync.dma_start(out=out[trows, ecol:ecol + 512], in_=ost[:])
            ev.append(('free', evict))
            return ev

        def do_norm_half(n, pair, half, hb):
            """Normalize one head's rows (hb=0: A rows 0:64, hb=1: B rows
            64:128) for one (pair, half).  Split so the A half can run at
            unit end, overlapping the deferred B AV passes."""
            attnp = attn0 if pair == 0 else attn1
            lo = n * L + half * 1024
            seg = ((n * 2 + pair) * 2 + half) * 2048 + hb * 1024
            packed = scratch.tile([P, 8], F32, name="packed", tag="packed")
            nc.sync.dma_start(
                out=packed[:],
                in_=denombuf[0:1, seg:seg + 1024].rearrange("a (p c) -> (a p) c", p=P))
            recp = scratch.tile([P, 8], F32, name="recp", tag="recp")
            nc.vector.reciprocal(recp[:], packed[:])
            recb = scratch.tile([P, 8], BF16, name="recb", tag="recb")
            nc.vector.tensor_copy(recb[:], recp[:])
            nc.sync.dma_start(
                out=recipbuf[0:1, seg:seg + 1024].rearrange("a (p c) -> (a p) c", p=P),
                in_=recb[:])
            rows = slice(0, 64) if hb == 0 else slice(64, 128)
            bct = scratch.tile([P, 1024], BF16, name="bct", tag="bct")
            nc.sync.dma_start(out=bct[rows, :],
                              in_=pbcast(recipbuf[0:1, seg:seg + 1024], 64))
            nc.vector.tensor_mul(attnp[rows, lo:lo + 1024], attnp[rows, lo:lo + 1024],
                                 bct[rows, :])

        def ev_normB(n, pair, half):
            return [('free', lambda: do_norm_half(n, pair, half, 1))]

        # ---------------- Phase 2 unit with aux interleave ----------------
        def p2_unit(n, pair, half, ps_s, ps_avA, aux, sink=None):
            QTp = QT0 if pair == 0 else QT1
            attnp = attn0 if pair == 0 else attn1
            lo = n * L + half * 1024
            seg = ((n * 2 + pair) * 2 + half) * 2048
            avA = ps_avA.tile([65, 1024], F32, tag="avA")
            ebs = []
            for mc in range(16):
                mo = n * L + mc * P
                sA = ps_s.tile([P, 1024], F32, tag="sA")
                sB = ps_s.tile([P, 1024], F32, tag="sB")
                for lq in range(2):
                    lc2 = slice(lo + lq * 512, lo + lq * 512 + 512)
                    nc.tensor.matmul(sA[:, lq * 512:(lq + 1) * 512],
                                     lhsT=KVT[0:64, mo:mo + P],
                                     rhs=QTp[0:64, lc2])
                    nc.tensor.matmul(sB[:, lq * 512:(lq + 1) * 512],
                                     lhsT=KTdup[64:128, mo:mo + P],
                                     rhs=QTp[64:128, lc2])
                eA = expA.tile([P, 1024], BF16, tag="eA")
                eB = expB.tile([P, 1024], BF16, tag="eB")
                nc.scalar.activation(eA[:], sA[:], mybir.ActivationFunctionType.Exp,
                                     scale=SCALE)
                nc.scalar.activation(eB[:], sB[:], mybir.ActivationFunctionType.Exp,
                                     scale=SCALE)
                ebs.append(eB)
                vw = Vp[n][:, mc * 80:mc * 80 + 65]
                for lq in range(2):
                    nc.tensor.matmul(avA[:, lq * 512:(lq + 1) * 512], lhsT=vw,
                                     rhs=eA[:, lq * 512:(lq + 1) * 512],
                                     start=mc == 0, stop=mc == 15)
                aux.pop(5)
            # head-A rows + denominator evict; head-B AV deferred to aux.
            # A-half normalization runs now, overlapping the deferred B work.
            nc.vector.tensor_copy(attnp[0:64, lo:lo + 1024], avA[0:64, :])
            dnA = scratch.tile([1, 1024], F32, tag="dnA")
            nc.vector.tensor_copy(dnA[:], avA[64:65, :])
            nc.sync.dma_start(out=denombuf[0:1, seg:seg + 1024], in_=dnA[:])
            do_norm_half(n, pair, half, 0)
            ev = (item_avB(n, pair, half, 0, ebs)
                  + item_avB(n, pair, half, 1, ebs)
                  + ev_normB(n, pair, half))
            if sink is None:
                aux.push_front(ev)
            else:
                sink.extend(ev)

        # ================= schedule =================
        def qb_kvb():
            nc.sync.dma_start(out=qb_sb[:], in_=qb)
            nc.sync.dma_start(out=kvb_sb[:], in_=kvb)

        aux = AuxQueue()
        # PE warmup: dependency-free matmuls on a zeroed tile keep the
        # Tensor engine busy while the first qt/weight DMAs land, so the
        # HAM clock governor reaches full speed before real work starts.
        with tc.tile_pool(name="warm", bufs=1, space="PSUM") as wps:
            wsb = consts.tile([P, 512], BF16)
            nc.vector.memset(wsb[:], 0)
            wt = wps.tile([P, 512], F32)
            for _ in range(8):
                nc.tensor.matmul(wt[:], lhsT=wsb[:, 0:P], rhs=wsb[:],
                                 start=True, stop=True)
        with tc.tile_pool(name="ps1a", bufs=1, space="PSUM") as ps1a, \
                tc.tile_pool(name="ps1kv", bufs=2, space="PSUM") as ps1kv:
            p1_tchunk(0, ps1a, ps1kv,
                      hook=lambda ec: (load_weights(ec),
                                       qb_kvb() if ec == 0 else None,
                                       late_consts() if ec == 6 else None))
            p1_tchunk(1, ps1a, ps1kv)
            load_opT()

        def push_p3(tts):
            for tt in tts:
                for eo in range(2):
                    for lq in range(2):
                        aux.push(item_p3(tt, eo, lq))

        pending = []  # events deferred from n0's last unit into n1's queue
        for n in range(N):
            if n == 0:
                with tc.tile_pool(name="ps_aux0", bufs=1, space="PSUM") as psx:
                    CUR['apool'] = AuxPool(psx, ["aux", "aux2"])
                    with tc.tile_pool(name="ps_s0", bufs=1, space="PSUM") as ps_s, \
                            tc.tile_pool(name="ps_avA0", bufs=1, space="PSUM") as ps_avA:
                        for tchunk in (2, 3):
                            for colhalf in range(2):
                                aux.push(item_p1_super(tchunk, colhalf))
                        p2_unit(n, 0, 0, ps_s, ps_avA, aux)
                        p2_unit(n, 0, 1, ps_s, ps_avA, aux)
                        p2_unit(n, 1, 0, ps_s, ps_avA, aux)
                        # tt0-3 ready once (p1,h0)'s norm (queued just above)
                        # lands; keeps the queue fed through the last unit
                        push_p3(range(0, 4))
                        p2_unit(n, 1, 1, ps_s, ps_avA, aux, sink=pending)
                    # leftovers drain at PE speed over 6 fresh banks
                    with tc.tile_pool(name="ps_gap0", bufs=1, space="PSUM") as gapp:
                        CUR['apool'] = AuxPool(
                            gapp, ["g0", "g1", "g2", "g3", "g4", "g5"])
                        aux.drain()
            else:
                with tc.tile_pool(name="ps_aux1", bufs=1, space="PSUM") as psx:
                    CUR['apool'] = AuxPool(psx, ["aux", "aux2"])
                    aux.push(pending)
                    # P3 for batch 0 rides the queue inside P2(n1)
                    push_p3(range(4, 16))
                    with tc.tile_pool(name="ps_s1", bufs=1, space="PSUM") as ps_s, \
                            tc.tile_pool(name="ps_avA1", bufs=1, space="PSUM") as ps_avA:
                        # half0 for both pairs first so its norms complete
                        # early and P3(n1, half0) can ride the queue
                        p2_unit(n, 0, 0, ps_s, ps_avA, aux)
                        p2_unit(n, 1, 0, ps_s, ps_avA, aux)
                        push_p3(range(16, 22))
                        p2_unit(n, 0, 1, ps_s, ps_avA, aux)
                        p2_unit(n, 1, 1, ps_s, ps_avA, aux)
                    # score/avA pools closed: 6 banks free for the drain.
                    # tt22-23 (half0, already normalized) cover the PE while
                    # the last B-half norm's DMA chain round-trips.
                    with tc.tile_pool(name="ps_tail", bufs=1, space="PSUM") as tailp:
                        CUR['apool'] = AuxPool(
                            tailp, ["t0", "t1", "t2", "t3", "t4", "t5"])
                        flip = [0]

                        def alt():
                            flip[0] ^= 1
                            return 'dve' if flip[0] else 'act'
                        for tt in range(22, 32):
                            for eo in range(2):
                                for lq in range(2):
                                    aux.push(item_p3(tt, eo, lq, alt()))
                        aux.drain()

    nc.compile()
    return nc


_NC_CACHE = None


def _get_nc():
    global _NC_CACHE
    if _NC_CACHE is None:
        _NC_CACHE = build_nc()
    return _NC_CACHE


def make_in_maps(query, q_proj, q_bias, kv_proj, kv_bias, out_proj):
    """Host-side sharding. Returns list of 8 per-core input dicts."""
    qT_h = np.ascontiguousarray(
        np.asarray(query, dtype=np.float32).transpose(2, 1, 0).reshape(E, T)
    ).astype(ml_dtypes.bfloat16)
    q_proj = np.asarray(q_proj, dtype=np.float32)
    q_bias = np.asarray(q_bias, dtype=np.float32)
    kv_proj = np.asarray(kv_proj, dtype=np.float32)
    kv_bias = np.asarray(kv_bias, dtype=np.float32)
    out_proj = np.asarray(out_proj, dtype=np.float32)

    in_maps = []
    for c in range(8):
        h0 = c // 2
        gis = range(4) if c % 2 == 0 else range(4, 8)
        rows_q = np.array([gi * (H * D) + h0 * D + d for gi in gis for d in range(D)])
        kv_rows = slice(h0 * 2 * D, (h0 + 1) * 2 * D)
        qpT_h = np.ascontiguousarray(
            q_proj[rows_q, :].T.reshape(16, P, DQ).transpose(1, 0, 2).reshape(P, 16 * DQ))
        kvpT_h = np.ascontiguousarray(
            kv_proj[kv_rows, :].T.reshape(16, P, P).transpose(1, 0, 2).reshape(P, 16 * P))
        in_maps.append({
            "qT": qT_h,
            "qpT": qpT_h.astype(ml_dtypes.bfloat16),
            "kvpT": kvpT_h.astype(ml_dtypes.bfloat16),
            "opT": np.ascontiguousarray(out_proj[:, rows_q].T).astype(ml_dtypes.bfloat16),
            "qb": np.ascontiguousarray(q_bias[rows_q].reshape(2, P).T),
            "kvb": np.ascontiguousarray(kv_bias[kv_rows].reshape(P, 1)),
            "ones16": np.ones((P, 16), dtype=ml_dtypes.bfloat16),
        })
    return in_maps


def kernel(query, q_proj, q_bias, kv_proj, kv_bias, out_proj, out_bias):
    from concourse.bass_utils import run_bass_kernel_spmd

    nc = _get_nc()
    in_maps = make_in_maps(query, q_proj, q_bias, kv_proj, kv_bias, out_proj)
    res = run_bass_kernel_spmd(nc, in_maps, core_ids=list(range(8)))
    total = np.zeros((T, E), dtype=np.float64)
    for rmap in res.results:
        total += rmap["out"].astype(np.float64)
    total += np.asarray(out_bias, dtype=np.float64)[None, :]
    return np.ascontiguousarray(
        total.reshape(N, L, E).transpose(1, 0, 2)).astype(np.float32)
